# revision 11
# baseline (speedup 1.0000x reference)
"""ContactMapHead Trainium2 kernel (8-way sharded, Bass/Tile).

Problem shapes (hardcoded): B=2, L=401, D=128.

Math (reference):
  pair[b,i,j,k] = x[b,i,:] @ W_bil[k] @ x[b,j,:] + b_bil[k]
  h  = LayerNorm_k(pair) * ln_g + ln_b
  y  = GELU(h @ W1.T + b1)
  contact[b,i,j] = y @ w2 + b2 ;  out = 0.5*(contact + contact^T)

Host folding:
  - W_bil centered over k  -> pair mean over k == 0, so LN = pair * rsqrt(mean(pair^2)+eps)
  - Wg[k,e] = W1[e,k]*ln_g[k],  cvec[e] = W1 @ ln_b + b1  (rides the GELU bias)
  - b2 and the symmetrization are applied on host (O(L^2), trivial)

Sharding: row-parallel. Core c (of 8) handles batch c//4, rows s..s+101
(s in {0,100,200,300}), padded to M=104 rows.

Device pipeline per core, (k x j) layout, bf16 matmuls, two passes so the
ScalarE activation table never thrashes between Sqrt and Gelu.

v2 restructure (trace-driven):
  - 4-row groups everywhere: one [128, 2048] PSUM tile (4 banks) per group,
    double-buffered (8 banks total).
  - pair^2 computed on ScalarE via ACTIVATE Square straight from PSUM with
    the bilinear bias folded in (the old DVE self-multiply ran at 2 cyc/elem
    from an SBUF port conflict).
  - The biased bf16 pair copy is one 4-row tensor_scalar on DVE.
  - var / w2-reduction matmuls write into the already-consumed first bank of
    the group's PSUM tile (no extra PSUM pool), and their tiny 4-row results
    are DMA'd straight out of PSUM (no PSUM->SBUF copy instructions).
  - Input W is spread across all five engine DMA queues; srep broadcasts
    alternate between the sync and gpsimd queues.
"""

import numpy as np
import ml_dtypes

import concourse.bass as bass
import concourse.tile as tile
from concourse import bacc, mybir

B, L, D = 2, 401, 128
NCORES = 8
GROUP = 4
NG_FULL = 26
M_FULL = GROUP * NG_FULL  # 104 padded rows per core
ROWS_VALID = 101
STARTS = (0, 100, 200, 300)

BF16 = mybir.dt.bfloat16
F32 = mybir.dt.float32
npbf16 = ml_dtypes.bfloat16
AF = mybir.ActivationFunctionType
ALU = mybir.AluOpType

RS = 416  # row stride (elements) inside 4-row SBUF tiles (4B-aligned bf16)


def _p32(ap_tile, nrows, ncols):
    """View of a [128, ...] tile exposing rows at partitions 0,32,64,96."""
    v = ap_tile.rearrange("(a b) f -> a b f", b=32)
    return v[:nrows, 0, :ncols]


def _rows(t, nrows, cols, stride):
    """(128, nrows, cols) view of a multi-row tile with given row stride."""
    v = t.rearrange("p (r c) -> p r c", c=stride)
    return v[:, :nrows, :cols]


def kernel_body(tc, ins, out_ap, ngroups):
    nc = tc.nc
    m = GROUP * ngroups
    from contextlib import ExitStack

    with ExitStack() as es:
        consts = es.enter_context(tc.tile_pool(name="consts", bufs=1))
        sb = es.enter_context(tc.tile_pool(name="sb", bufs=3))
        sbh = es.enter_context(tc.tile_pool(name="sbh", bufs=3))
        # One PSUM pool: [128, 2048] fp32 = 4 banks, double-buffered = all 8.
        ps = es.enter_context(tc.tile_pool(name="ps", bufs=2, space="PSUM"))
        dram = es.enter_context(tc.tile_pool(name="dram", bufs=1, space="DRAM"))

        # ---- constants / inputs to SBUF (small ones first, on idle queues)
        xT_sb = consts.tile([128, L], BF16, tag="xT")
        nc.sync.dma_start(out=xT_sb, in_=ins["xT"])
        XcT_sb = consts.tile([128, m], BF16, tag="XcT")
        nc.sync.dma_start(out=XcT_sb, in_=ins["XcT"])
        bsh_sb = consts.tile([128, 1], F32, tag="bsh")
        nc.gpsimd.dma_start(out=bsh_sb, in_=ins["bsh"])
        cvec_sb = consts.tile([128, 1], F32, tag="cvec")
        nc.gpsimd.dma_start(out=cvec_sb, in_=ins["cvec"])
        Wg_sb = consts.tile([128, D], BF16, tag="Wg")
        nc.gpsimd.dma_start(out=Wg_sb, in_=ins["Wg"])
        ones_sb = consts.tile([128, 32], BF16, tag="ones")
        nc.vector.memset(ones_sb, 1.0)
        w2r_sb = consts.tile([128, 32], BF16, tag="w2r")
        nc.gpsimd.dma_start(out=w2r_sb, in_=ins["w2"])
        eps_sb = consts.tile([128, 1], F32, tag="eps")
        nc.vector.memset(eps_sb, 1e-5)

        # W (d x k*e), 8 chunks spread over the 5 engine DMA queues so step0
        # can start on chunk 0 while the rest stream in.
        wpool = tc.alloc_tile_pool(name="wpool", bufs=1)
        WCH = 2048
        w_queues = [nc.sync, nc.gpsimd, nc.scalar]
        W_t = []
        for c in range(D * D // WCH):
            wt = wpool.tile([128, WCH], BF16, tag=f"W{c}", name=f"W{c}")
            w_queues[c % len(w_queues)].dma_start(
                out=wt, in_=ins["W"][:, c * WCH : (c + 1) * WCH]
            )
            W_t.append(wt)

        T_sb = consts.tile([128, D * m], BF16, tag="T")  # T[e, k*m + i]

        # stats staging
        S_all = consts.tile([128, L], F32, tag="Sall")  # rows 0..m-1: var -> s
        S_bf = consts.tile([128, L], BF16, tag="Sbf")
        s_dram = dram.tile([m, L], BF16, tag="sdram")

        # ---- step 0: tmp[i,k,e] for all rows
        KB = max(1, 512 // m)  # k's per psum 512-block
        while D % KB:
            KB -= 1
        BLKS = 4  # 512-blocks per [128, 2048] tile
        for t4 in range(D // (KB * BLKS)):
            ps0 = ps.tile([128, 2048], F32, tag="big")
            for blk in range(BLKS):
                for kk in range(KB):
                    k = (t4 * BLKS + blk) * KB + kk
                    c, off = divmod(k * D, WCH)
                    nc.tensor.matmul(
                        ps0[:, 512 * blk + kk * m : 512 * blk + kk * m + m],
                        W_t[c][:, off : off + D],
                        XcT_sb[:, :m],
                        start=True,
                        stop=True,
                    )
            # copy all 4 blocks in one strided op, alternating engines
            v = ps0.rearrange("p (h c) -> p h c", c=512)[:, :BLKS, : KB * m]
            dstv = T_sb[:, t4 * BLKS * KB * m : (t4 + 1) * BLKS * KB * m]
            dstv = dstv.rearrange("p (h c) -> p h c", c=KB * m)
            if t4 % 2 == 0:
                nc.scalar.activation(dstv, v, AF.Copy)
            else:
                nc.vector.tensor_copy(dstv, v)

        # pair rows persist across the two passes (alloc after W release)
        wpool.release()
        prowp = es.enter_context(tc.tile_pool(name="prowp", bufs=1))
        prow = [
            prowp.tile([128, GROUP * RS], BF16, tag=f"prow{g}", name=f"prow{g}")
            for g in range(ngroups)
        ]

        T_k_i = T_sb.rearrange("p (k i) -> p k i", i=m)

        # ---- pass A: pair rows + variance (software-pipelined: the Tensor
        # queue gets group g+1's pair matmuls BEFORE group g's var matmuls,
        # so the PE never head-of-line blocks on the S/V elementwise ops).
        psA_t = {}

        def pairmm(g):
            psA_t[g] = ps.tile([128, 2048], F32, tag="big", name=f"psA{g}")
            for r in range(GROUP):
                i = GROUP * g + r
                nc.tensor.matmul(
                    psA_t[g][:, 512 * r : 512 * r + L],
                    T_k_i[:, :, i],
                    xT_sb[:, :L],
                    start=True,
                    stop=True,
                )

        sq_t = {}

        def elemwiseA(g):
            psA = psA_t[g]
            psv = _rows(psA, GROUP, L, 512)
            # ScalarE: pair2 = (pair0 + bsh)^2 straight from PSUM
            pair2 = sb.tile([128, GROUP * RS], BF16, tag="pair2")
            nc.scalar.activation(
                _rows(pair2, GROUP, L, RS), psv, AF.Square, bias=bsh_sb, scale=1.0
            )
            sq_t[g] = pair2
            # DVE: biased bf16 pair copy for pass B
            nc.vector.tensor_scalar(
                _rows(prow[g], GROUP, L, RS), psv, bsh_sb, None, op0=ALU.add
            )

        def varmm(g):
            psA, pair2 = psA_t[g], sq_t[g]
            for r in range(GROUP):
                nc.tensor.matmul(
                    psA[32 * r : 32 * (r + 1), :L],
                    ones_sb,
                    pair2[:, RS * r : RS * r + L],
                    start=True,
                    stop=True,
                    tile_position=(0, 32 * r),
                )

        def varout(g):
            psA = psA_t.pop(g)
            sq_t.pop(g)
            vstage = sb.tile([128, L], F32, tag="vstage")
            nc.scalar.activation(vstage[:, :L], psA[:, :L], AF.Copy)
            nc.gpsimd.dma_start(
                out=S_all[GROUP * g : GROUP * (g + 1), :L],
                in_=_p32(vstage, GROUP, L),
            )

        pairmm(0)
        for g in range(ngroups):
            if g + 1 < ngroups:
                pairmm(g + 1)
            elemwiseA(g)
            varmm(g)
            varout(g)

        # ---- stats (once): std = sqrt(var/128 + eps); s = 1/std; to DRAM
        nc.scalar.activation(
            S_all[:m, :L], S_all[:m, :L], AF.Sqrt, bias=eps_sb[:m], scale=1.0 / D
        )
        nc.vector.reciprocal_approx_fast(S_all[:m, :L], S_all[:m, :L])
        nc.vector.tensor_copy(S_bf[:m, :L], S_all[:m, :L])
        nc.sync.dma_start(out=s_dram[:, :], in_=S_bf[:m, :L])

        # ---- pass B: scale, MLP, contact rows (w2 matmuls lag one group so
        # the Tensor queue never waits on the gelu of the same group).
        psB_t, h2_t = {}, {}

        def srep_dma(g):
            srep = sbh.tile([128, GROUP * L], BF16, tag="srep")
            rows = s_dram[GROUP * g : GROUP * (g + 1), :]
            bcast = bass.AP(
                tensor=rows.tensor, offset=rows.offset, ap=[[0, 128], [1, GROUP * L]]
            )
            (nc.sync if g % 2 == 0 else nc.gpsimd).dma_start(out=srep, in_=bcast)
            return srep

        srep_t = {0: srep_dma(0), 1: srep_dma(1)}

        def mlp(g):
            # DVE: h = pair * s  (both bf16 SBUF -> 2x mode)
            srep = srep_t.pop(g)
            h = sbh.tile([128, GROUP * RS], BF16, tag="h")
            nc.vector.tensor_mul(
                _rows(h, GROUP, L, RS),
                _rows(prow[g], GROUP, L, RS),
                _rows(srep, GROUP, L, L),
            )
            psB_t[g] = ps.tile([128, 2048], F32, tag="big", name=f"psB{g}")
            for r in range(GROUP):
                nc.tensor.matmul(
                    psB_t[g][:, 512 * r : 512 * r + L],
                    Wg_sb,
                    h[:, RS * r : RS * r + L],
                    start=True,
                    stop=True,
                )

        def gelu(g):
            h2_t[g] = sbh.tile([128, GROUP * RS], BF16, tag="h2", name=f"h2_{g}")
            nc.scalar.activation(
                _rows(h2_t[g], GROUP, L, RS), _rows(psB_t[g], GROUP, L, 512),
                AF.Gelu, bias=cvec_sb, scale=1.0,
            )

        def w2out(g):
            psB, h2 = psB_t.pop(g), h2_t.pop(g)
            for r in range(GROUP):
                nc.tensor.matmul(
                    psB[32 * r : 32 * (r + 1), :L],
                    w2r_sb,
                    h2[:, RS * r : RS * r + L],
                    start=True,
                    stop=True,
                    tile_position=(0, 32 * r),
                )
            ostage = sb.tile([128, L], F32, tag="ostage")
            nc.vector.tensor_copy(ostage[:, :L], psB[:, :L])
            nc.gpsimd.dma_start(
                out=out_ap[GROUP * g : GROUP * (g + 1), :],
                in_=_p32(ostage, GROUP, L),
            )

        mlp(0)
        gelu(0)
        for g in range(1, ngroups):
            if g + 1 < ngroups:
                srep_t[g + 1] = srep_dma(g + 1)
            mlp(g)
            w2out(g - 1)
            gelu(g)
        w2out(ngroups - 1)


def build_nc(ngroups=NG_FULL):
    m = GROUP * ngroups
    nc = bacc.Bacc("TRN2", debug=False)
    ins = {
        "xT": nc.dram_tensor("xT", [D, L], BF16, kind="ExternalInput").ap(),
        "XcT": nc.dram_tensor("XcT", [D, m], BF16, kind="ExternalInput").ap(),
        "W": nc.dram_tensor("W", [D, D * D], BF16, kind="ExternalInput").ap(),
        "bsh": nc.dram_tensor("bsh", [D, 1], F32, kind="ExternalInput").ap(),
        "cvec": nc.dram_tensor("cvec", [D, 1], F32, kind="ExternalInput").ap(),
        "Wg": nc.dram_tensor("Wg", [D, D], BF16, kind="ExternalInput").ap(),
        "w2": nc.dram_tensor("w2", [D, 32], BF16, kind="ExternalInput").ap(),
    }
    out = nc.dram_tensor("out", [m, L], F32, kind="ExternalOutput").ap()
    with tile.TileContext(nc) as tc:
        kernel_body(tc, ins, out, ngroups)
    nc.compile()
    return nc


def host_prep(x, W_bil, b_bil, ln_g, ln_b, W1, b1, w2, b2):
    """Fold weights on host; build the 8 per-core input maps."""
    x = np.asarray(x, np.float32)
    W_bil = np.asarray(W_bil, np.float32)
    b_bil = np.asarray(b_bil, np.float32)
    ln_g = np.asarray(ln_g, np.float32)
    ln_b = np.asarray(ln_b, np.float32)
    W1 = np.asarray(W1, np.float32)
    b1 = np.asarray(b1, np.float32)
    w2 = np.asarray(w2, np.float32)

    Wc = W_bil - W_bil.mean(axis=0, keepdims=True)  # (k,d,e)
    W_host = np.ascontiguousarray(Wc.transpose(1, 0, 2).reshape(D, D * D)).astype(
        npbf16
    )
    bsh = (b_bil - b_bil.mean()).reshape(D, 1).astype(np.float32)
    Wg = np.ascontiguousarray((W1 * ln_g[None, :]).T).astype(npbf16)  # (k, e2)
    cvec = (W1 @ ln_b + b1).reshape(D, 1).astype(np.float32)
    w2c = np.ascontiguousarray(np.repeat(w2.reshape(D, 1), 32, axis=1)).astype(npbf16)

    xT = [np.ascontiguousarray(x[b].T).astype(npbf16) for b in range(B)]  # (D, L)

    in_maps = []
    for c in range(NCORES):
        b, s = c // 4, STARTS[c % 4]
        xc = np.zeros((M_FULL, D), np.float32)
        xc[:ROWS_VALID] = x[b, s : s + ROWS_VALID]
        in_maps.append(
            {
                "xT": xT[b],
                "XcT": np.ascontiguousarray(xc.T).astype(npbf16),
                "W": W_host,
                "bsh": bsh,
                "cvec": cvec,
                "Wg": Wg,
                "w2": w2c,
            }
        )
    return in_maps


def assemble(results, b2):
    """Gather per-core row blocks into the full symmetrized output."""
    contact = np.empty((B, L, L), np.float32)
    for c in range(NCORES):
        b, s = c // 4, STARTS[c % 4]
        contact[b, s : s + ROWS_VALID, :] = results[c]["out"][:ROWS_VALID]
    contact += np.float32(np.asarray(b2, np.float32).reshape(-1)[0])
    return (0.5 * (contact + contact.transpose(0, 2, 1))).astype(np.float32)


_NC_CACHE = {}


def _get_nc():
    if "nc" not in _NC_CACHE:
        _NC_CACHE["nc"] = build_nc(NG_FULL)
    return _NC_CACHE["nc"]


def run_on_device(in_maps, trace=False):
    from concourse.bass_utils import run_bass_kernel_spmd

    nc = _get_nc()
    return run_bass_kernel_spmd(
        nc, in_maps, core_ids=list(range(NCORES)), trace=trace
    )


def kernel(x, W_bil, b_bil, ln_g, ln_b, W1, b1, w2, b2):
    in_maps = host_prep(x, W_bil, b_bil, ln_g, ln_b, W1, b1, w2, b2)
    res = run_on_device(in_maps, trace=False)
    return assemble(res.results, b2)


# revision 19
# speedup vs baseline: 1.1478x; 1.1478x over previous
"""ContactMapHead Trainium2 kernel (8-way sharded, Bass/Tile).

Problem shapes (hardcoded): B=2, L=401, D=128.

Math (reference):
  pair[b,i,j,k] = x[b,i,:] @ W_bil[k] @ x[b,j,:] + b_bil[k]
  h  = LayerNorm_k(pair) * ln_g + ln_b
  y  = GELU(h @ W1.T + b1)
  contact[b,i,j] = y @ w2 + b2 ;  out = 0.5*(contact + contact^T)

Host folding:
  - W_bil centered over k  -> pair mean over k == 0, so LN = pair * rsqrt(mean(pair^2)+eps)
  - Wg[k,e] = W1[e,k]*ln_g[k],  cvec[e] = W1 @ ln_b + b1  (rides the GELU bias)
  - b2 and the symmetrization are applied on host (O(L^2), trivial)

Sharding: row-parallel. Core c (of 8) handles batch c//4, rows s..s+101
(s in {0,100,200,300}), padded to M=104 rows.

Device pipeline per core, (k x j) layout, bf16 matmuls, two passes so the
ScalarE activation table never thrashes between Sqrt and Gelu. Elementwise
ops batch TWO rows per instruction (sequencer issue overhead dominates at
one-row granularity). PSUM->SBUF copies are split ScalarE-front/DVE-back.
M=1 reduction matmuls run as M=32 col-strip tiles so PSUM partitions are
fully initialized for full-tile copies.
"""

import numpy as np
import ml_dtypes

import concourse.bass as bass
import concourse.tile as tile
from concourse import bacc, mybir

B, L, D = 2, 401, 128
NCORES = 8
GROUP = 4
NG_FULL = 26
M_FULL = GROUP * NG_FULL  # 104 padded rows per core
ROWS_VALID = 101
STARTS = (0, 100, 200, 300)

BF16 = mybir.dt.bfloat16
F32 = mybir.dt.float32
npbf16 = ml_dtypes.bfloat16
AF = mybir.ActivationFunctionType
ALU = mybir.AluOpType

RS = 416  # row stride (elements) inside two-row SBUF tiles (4B-aligned bf16)
LH = 208  # ScalarE/DVE split point for PSUM->SBUF copies


def _p32(ap_tile, nrows, ncols):
    """View of a [128, ...] tile exposing rows at partitions 0,32,64,96."""
    v = ap_tile.rearrange("(a b) f -> a b f", b=32)
    return v[:nrows, 0, :ncols]


def _rows2(t, cols, stride):
    """(128, 2, cols) view of a two-row tile with given row stride."""
    v = t.rearrange("p (r c) -> p r c", c=stride)
    return v[:, :2, :cols]


def kernel_body(tc, ins, out_ap, ngroups):
    nc = tc.nc
    m = GROUP * ngroups
    npairs = m // 2
    from contextlib import ExitStack

    with ExitStack() as es:
        consts = es.enter_context(tc.tile_pool(name="consts", bufs=1))
        sb = es.enter_context(tc.tile_pool(name="sb", bufs=4))
        sbh = es.enter_context(tc.tile_pool(name="sbh", bufs=4))
        ps_big = es.enter_context(tc.tile_pool(name="ps_big", bufs=3, space="PSUM"))
        ps_small = es.enter_context(tc.tile_pool(name="ps_small", bufs=2, space="PSUM"))
        dram = es.enter_context(tc.tile_pool(name="dram", bufs=1, space="DRAM"))

        # ---- constants / inputs to SBUF
        xT_sb = consts.tile([128, L], BF16, tag="xT")
        nc.sync.dma_start(out=xT_sb, in_=ins["xT"])
        XcT_sb = consts.tile([128, m], BF16, tag="XcT")
        nc.sync.dma_start(out=XcT_sb, in_=ins["XcT"])
        bsh_sb = consts.tile([128, 1], F32, tag="bsh")
        nc.sync.dma_start(out=bsh_sb, in_=ins["bsh"])
        cvec_sb = consts.tile([128, 1], F32, tag="cvec")
        nc.sync.dma_start(out=cvec_sb, in_=ins["cvec"])
        Wg_sb = consts.tile([128, D], BF16, tag="Wg")
        nc.sync.dma_start(out=Wg_sb, in_=ins["Wg"])
        ones_sb = consts.tile([128, 32], BF16, tag="ones")
        nc.vector.memset(ones_sb, 1.0)
        w2r_sb = consts.tile([128, 32], BF16, tag="w2r")
        nc.sync.dma_start(out=w2r_sb, in_=ins["w2"])
        eps_sb = consts.tile([128, 1], F32, tag="eps")
        nc.vector.memset(eps_sb, 1e-5)

        # W (d x k*e), 8 chunks so step0 can start before the full 4MB lands.
        # Own pool (created last = top of pool stack), released after step0
        # to give the SBUF back.
        wpool = tc.alloc_tile_pool(name="wpool", bufs=1)
        WCH = 2048
        W_t = []
        for c in range(D * D // WCH):
            wt = wpool.tile([128, WCH], BF16, tag=f"W{c}", name=f"W{c}")
            nc.sync.dma_start(out=wt, in_=ins["W"][:, c * WCH : (c + 1) * WCH])
            W_t.append(wt)

        T_sb = consts.tile([128, D * m], BF16, tag="T")  # T[e, k*m + i]

        # stats staging
        S_all = consts.tile([128, L], F32, tag="Sall")  # rows 0..m-1: var -> s
        S_bf = consts.tile([128, L], BF16, tag="Sbf")
        s_dram = dram.tile([m, L], BF16, tag="sdram")

        # ---- step 0: tmp[i,k,e] for all rows
        KB = max(1, 512 // m)  # k's per psum bank
        while D % KB:
            KB -= 1
        BANKS2 = 1024 // 512  # big tiles are 2 banks
        for kb2 in range(D // (KB * BANKS2)):
            ps0 = ps_big.tile([128, 1024], F32, tag="big")
            for half in range(BANKS2):
                kb = kb2 * BANKS2 + half
                for kk in range(KB):
                    k = kb * KB + kk
                    c, off = divmod(k * D, WCH)
                    nc.tensor.matmul(
                        ps0[:, 512 * half + kk * m : 512 * half + kk * m + m],
                        W_t[c][:, off : off + D],
                        XcT_sb[:, :m],
                        start=True,
                        stop=True,
                    )
            # copy both banks in one strided op per engine
            v = ps0.rearrange("p (h c) -> p h c", c=512)[:, :2, : KB * m]
            dstv = T_sb[:, kb2 * BANKS2 * KB * m : (kb2 + 1) * BANKS2 * KB * m]
            dstv = dstv.rearrange("p (h c) -> p h c", c=KB * m)
            if kb2 % 2 == 0:
                nc.scalar.activation(dstv, v, AF.Copy)
            else:
                nc.vector.tensor_copy(dstv, v)

        # pair_c row-pairs persist across the two passes (alloc after W release)
        wpool.release()
        prowp = es.enter_context(tc.tile_pool(name="prowp", bufs=1))
        prow2 = [
            prowp.tile([128, 2 * RS], BF16, tag=f"prow{i}", name=f"prow{i}")
            for i in range(npairs)
        ]

        T_k_i = T_sb.rearrange("p (k i) -> p k i", i=m)

        def split_copy2(dst2, src_ps2, bias=None):
            """Two-row PSUM->SBUF copy (+optional per-partition bias),
            front columns on ScalarE, back on DVE."""
            sv = _rows2(src_ps2, L, 512)
            dv = _rows2(dst2, L, RS)
            if bias is not None:
                nc.scalar.activation(
                    dv[:, :, :LH], sv[:, :, :LH], AF.Identity, bias=bias, scale=1.0
                )
                nc.vector.tensor_scalar(
                    dv[:, :, LH:L], sv[:, :, LH:L], bias, None, op0=ALU.add
                )
            else:
                nc.scalar.activation(dv[:, :, :LH], sv[:, :, :LH], AF.Copy)
                nc.vector.tensor_copy(dv[:, :, LH:L], sv[:, :, LH:L])

        # ---- pass A: pair rows + variance
        for g in range(ngroups):
            var_ps = ps_small.tile([128, 512], F32, tag="small")
            pair_ps2 = []
            for p in range(2):
                pp = ps_big.tile([128, 1024], F32, tag="big")
                for r in range(2):
                    i = GROUP * g + 2 * p + r
                    nc.tensor.matmul(
                        pp[:, 512 * r : 512 * r + L],
                        T_k_i[:, :, i],
                        xT_sb[:, :L],
                        start=True,
                        stop=True,
                    )
                pair_ps2.append(pp)
            pair22 = []
            for p in range(2):
                i2 = 2 * g + p
                split_copy2(prow2[i2], pair_ps2[p], bias=bsh_sb)
                p2 = sb.tile([128, 2 * RS], BF16, tag="pair2")
                if p == 0:
                    nc.gpsimd.tensor_mul(
                        _rows2(p2, L, RS), _rows2(prow2[i2], L, RS),
                        _rows2(prow2[i2], L, RS),
                    )
                else:
                    nc.vector.tensor_mul(
                        _rows2(p2, L, RS), _rows2(prow2[i2], L, RS),
                        _rows2(prow2[i2], L, RS),
                    )
                pair22.append(p2)
            for r in range(GROUP):
                nc.tensor.matmul(
                    var_ps[32 * r : 32 * (r + 1), :L],
                    ones_sb,
                    pair22[r // 2][:, RS * (r % 2) : RS * (r % 2) + L],
                    start=True,
                    stop=True,
                    tile_position=(0, 32 * r),
                )
            var_sb = sb.tile([128, L], F32, tag="var_sb")
            nc.scalar.activation(var_sb[:, :LH], var_ps[:, :LH], AF.Copy)
            nc.vector.tensor_copy(var_sb[:, LH:L], var_ps[:, LH:L])
            nc.gpsimd.dma_start(
                out=S_all[GROUP * g : GROUP * (g + 1), :L],
                in_=_p32(var_sb, GROUP, L),
            )

        # ---- stats (once): std = sqrt(var/128 + eps); s = 1/std; to DRAM
        nc.scalar.activation(
            S_all[:m, :L], S_all[:m, :L], AF.Sqrt, bias=eps_sb[:m], scale=1.0 / D
        )
        nc.vector.reciprocal_approx_fast(S_all[:m, :L], S_all[:m, :L])
        nc.vector.tensor_copy(S_bf[:m, :L], S_all[:m, :L])
        nc.sync.dma_start(out=s_dram[:, :], in_=S_bf[:m, :L])

        # ---- pass B: scale, MLP, contact rows
        for g in range(ngroups):
            out_ps = ps_small.tile([128, 512], F32, tag="small")
            srep4 = sbh.tile([128, GROUP * L], BF16, tag="srep")
            rows = s_dram[GROUP * g : GROUP * (g + 1), :]
            bcast = bass.AP(
                tensor=rows.tensor, offset=rows.offset, ap=[[0, 128], [1, GROUP * L]]
            )
            nc.sync.dma_start(out=srep4, in_=bcast)
            h_tiles = []
            for p in range(2):
                i2 = 2 * g + p
                h = sbh.tile([128, 2 * RS], BF16, tag="h")
                sview = srep4.rearrange("q (r c) -> q r c", c=L)[:, 2 * p : 2 * p + 2, :]
                nc.vector.tensor_mul(_rows2(h, L, RS), _rows2(prow2[i2], L, RS), sview)
                h_tiles.append(h)
            y_ps2 = []
            for p in range(2):
                yp = ps_big.tile([128, 1024], F32, tag="big")
                for r in range(2):
                    nc.tensor.matmul(
                        yp[:, 512 * r : 512 * r + L],
                        Wg_sb,
                        h_tiles[p][:, RS * r : RS * r + L],
                        start=True,
                        stop=True,
                    )
                y_ps2.append(yp)
            h2_tiles = []
            for p in range(2):
                h2 = sbh.tile([128, 2 * RS], BF16, tag="h2")
                nc.scalar.activation(
                    _rows2(h2, L, RS), _rows2(y_ps2[p], L, 512), AF.Gelu,
                    bias=cvec_sb, scale=1.0,
                )
                h2_tiles.append(h2)
            for r in range(GROUP):
                nc.tensor.matmul(
                    out_ps[32 * r : 32 * (r + 1), :L],
                    w2r_sb,
                    h2_tiles[r // 2][:, RS * (r % 2) : RS * (r % 2) + L],
                    start=True,
                    stop=True,
                    tile_position=(0, 32 * r),
                )
            orow = sb.tile([128, L], F32, tag="orow")
            nc.scalar.activation(orow[:, :LH], out_ps[:, :LH], AF.Copy)
            nc.vector.tensor_copy(orow[:, LH:L], out_ps[:, LH:L])
            nc.scalar.dma_start(
                out=out_ap[GROUP * g : GROUP * (g + 1), :],
                in_=_p32(orow, GROUP, L),
            )


def build_nc(ngroups=NG_FULL):
    m = GROUP * ngroups
    nc = bacc.Bacc("TRN2", debug=False)
    ins = {
        "xT": nc.dram_tensor("xT", [D, L], BF16, kind="ExternalInput").ap(),
        "XcT": nc.dram_tensor("XcT", [D, m], BF16, kind="ExternalInput").ap(),
        "W": nc.dram_tensor("W", [D, D * D], BF16, kind="ExternalInput").ap(),
        "bsh": nc.dram_tensor("bsh", [D, 1], F32, kind="ExternalInput").ap(),
        "cvec": nc.dram_tensor("cvec", [D, 1], F32, kind="ExternalInput").ap(),
        "Wg": nc.dram_tensor("Wg", [D, D], BF16, kind="ExternalInput").ap(),
        "w2": nc.dram_tensor("w2", [D, 32], BF16, kind="ExternalInput").ap(),
    }
    out = nc.dram_tensor("out", [m, L], F32, kind="ExternalOutput").ap()
    with tile.TileContext(nc) as tc:
        kernel_body(tc, ins, out, ngroups)
    nc.compile()
    return nc


def host_prep(x, W_bil, b_bil, ln_g, ln_b, W1, b1, w2, b2):
    """Fold weights on host; build the 8 per-core input maps."""
    x = np.asarray(x, np.float32)
    W_bil = np.asarray(W_bil, np.float32)
    b_bil = np.asarray(b_bil, np.float32)
    ln_g = np.asarray(ln_g, np.float32)
    ln_b = np.asarray(ln_b, np.float32)
    W1 = np.asarray(W1, np.float32)
    b1 = np.asarray(b1, np.float32)
    w2 = np.asarray(w2, np.float32)

    Wc = W_bil - W_bil.mean(axis=0, keepdims=True)  # (k,d,e)
    W_host = np.ascontiguousarray(Wc.transpose(1, 0, 2).reshape(D, D * D)).astype(
        npbf16
    )
    bsh = (b_bil - b_bil.mean()).reshape(D, 1).astype(np.float32)
    Wg = np.ascontiguousarray((W1 * ln_g[None, :]).T).astype(npbf16)  # (k, e2)
    cvec = (W1 @ ln_b + b1).reshape(D, 1).astype(np.float32)
    w2c = np.ascontiguousarray(np.repeat(w2.reshape(D, 1), 32, axis=1)).astype(npbf16)

    xT = [np.ascontiguousarray(x[b].T).astype(npbf16) for b in range(B)]  # (D, L)

    in_maps = []
    for c in range(NCORES):
        b, s = c // 4, STARTS[c % 4]
        xc = np.zeros((M_FULL, D), np.float32)
        xc[:ROWS_VALID] = x[b, s : s + ROWS_VALID]
        in_maps.append(
            {
                "xT": xT[b],
                "XcT": np.ascontiguousarray(xc.T).astype(npbf16),
                "W": W_host,
                "bsh": bsh,
                "cvec": cvec,
                "Wg": Wg,
                "w2": w2c,
            }
        )
    return in_maps


def assemble(results, b2):
    """Gather per-core row blocks into the full symmetrized output."""
    contact = np.empty((B, L, L), np.float32)
    for c in range(NCORES):
        b, s = c // 4, STARTS[c % 4]
        contact[b, s : s + ROWS_VALID, :] = results[c]["out"][:ROWS_VALID]
    contact += np.float32(np.asarray(b2, np.float32).reshape(-1)[0])
    return (0.5 * (contact + contact.transpose(0, 2, 1))).astype(np.float32)


_NC_CACHE = {}


def _get_nc():
    if "nc" not in _NC_CACHE:
        _NC_CACHE["nc"] = build_nc(NG_FULL)
    return _NC_CACHE["nc"]


def run_on_device(in_maps, trace=False):
    from concourse.bass_utils import run_bass_kernel_spmd

    nc = _get_nc()
    return run_bass_kernel_spmd(
        nc, in_maps, core_ids=list(range(NCORES)), trace=trace
    )


def kernel(x, W_bil, b_bil, ln_g, ln_b, W1, b1, w2, b2):
    in_maps = host_prep(x, W_bil, b_bil, ln_g, ln_b, W1, b1, w2, b2)
    res = run_on_device(in_maps, trace=False)
    return assemble(res.results, b2)


# revision 20
# speedup vs baseline: 1.3922x; 1.2129x over previous
"""ContactMapHead Trainium2 kernel (8-way sharded, Bass/Tile).

Problem shapes (hardcoded): B=2, L=401, D=128.

Math (reference):
  pair[b,i,j,k] = x[b,i,:] @ W_bil[k] @ x[b,j,:] + b_bil[k]
  h  = LayerNorm_k(pair) * ln_g + ln_b
  y  = GELU(h @ W1.T + b1)
  contact[b,i,j] = y @ w2 + b2 ;  out = 0.5*(contact + contact^T)

Host folding:
  - W_bil centered over k  -> pair mean over k == 0, so LN = pair * rsqrt(mean(pair^2)+eps)
  - Wg[k,e] = W1[e,k]*ln_g[k],  cvec[e] = W1 @ ln_b + b1  (rides the GELU bias)
  - b2 and the symmetrization are applied on host (O(L^2), trivial)

Sharding: row-parallel. Core c (of 8) handles batch c//4, rows s..s+101
(s in {0,100,200,300}), padded to M=104 rows.

Device pipeline per core, (k x j) layout, bf16 matmuls, two passes so the
ScalarE activation table never thrashes between Sqrt and Gelu. Elementwise
ops batch TWO rows per instruction (sequencer issue overhead dominates at
one-row granularity). PSUM->SBUF copies are split ScalarE-front/DVE-back.
M=1 reduction matmuls run as M=32 col-strip tiles so PSUM partitions are
fully initialized for full-tile copies.
"""

import numpy as np
import ml_dtypes

import concourse.bass as bass
import concourse.tile as tile
from concourse import bacc, mybir

B, L, D = 2, 401, 128
NCORES = 8
GROUP = 4
NG_FULL = 26
M_FULL = GROUP * NG_FULL  # 104 padded rows per core
ROWS_VALID = 101
STARTS = (0, 100, 200, 300)

BF16 = mybir.dt.bfloat16
F32 = mybir.dt.float32
npbf16 = ml_dtypes.bfloat16
AF = mybir.ActivationFunctionType
ALU = mybir.AluOpType

RS = 416  # row stride (elements) inside two-row SBUF tiles (4B-aligned bf16)
LH = 208  # ScalarE/DVE split point for PSUM->SBUF copies


def _p32(ap_tile, nrows, ncols):
    """View of a [128, ...] tile exposing rows at partitions 0,32,64,96."""
    v = ap_tile.rearrange("(a b) f -> a b f", b=32)
    return v[:nrows, 0, :ncols]


def _rows2(t, cols, stride):
    """(128, 2, cols) view of a two-row tile with given row stride."""
    v = t.rearrange("p (r c) -> p r c", c=stride)
    return v[:, :2, :cols]


def kernel_body(tc, ins, out_ap, ngroups):
    nc = tc.nc
    m = GROUP * ngroups
    npairs = m // 2
    from contextlib import ExitStack

    with ExitStack() as es:
        consts = es.enter_context(tc.tile_pool(name="consts", bufs=1))
        sb = es.enter_context(tc.tile_pool(name="sb", bufs=4))
        sbh = es.enter_context(tc.tile_pool(name="sbh", bufs=4))
        ps_big = es.enter_context(tc.tile_pool(name="ps_big", bufs=3, space="PSUM"))
        ps_small = es.enter_context(tc.tile_pool(name="ps_small", bufs=2, space="PSUM"))
        dram = es.enter_context(tc.tile_pool(name="dram", bufs=1, space="DRAM"))

        # ---- constants / inputs to SBUF
        xT_sb = consts.tile([128, L], BF16, tag="xT")
        nc.sync.dma_start(out=xT_sb, in_=ins["xT"])
        XcT_sb = consts.tile([128, m], BF16, tag="XcT")
        nc.sync.dma_start(out=XcT_sb, in_=ins["XcT"])
        bsh_sb = consts.tile([128, 1], F32, tag="bsh")
        nc.sync.dma_start(out=bsh_sb, in_=ins["bsh"])
        cvec_sb = consts.tile([128, 1], F32, tag="cvec")
        nc.sync.dma_start(out=cvec_sb, in_=ins["cvec"])
        Wg_sb = consts.tile([128, D], BF16, tag="Wg")
        nc.sync.dma_start(out=Wg_sb, in_=ins["Wg"])
        ones_sb = consts.tile([128, 32], BF16, tag="ones")
        nc.vector.memset(ones_sb, 1.0)
        w2r_sb = consts.tile([128, 32], BF16, tag="w2r")
        nc.sync.dma_start(out=w2r_sb, in_=ins["w2"])
        eps_sb = consts.tile([128, 1], F32, tag="eps")
        nc.vector.memset(eps_sb, 1e-5)

        # W (d x k*e), 8 chunks so step0 can start before the full 4MB lands.
        # Own pool (created last = top of pool stack), released after step0
        # to give the SBUF back.
        wpool = tc.alloc_tile_pool(name="wpool", bufs=1)
        WCH = 2048
        W_t = []
        for c in range(D * D // WCH):
            wt = wpool.tile([128, WCH], BF16, tag=f"W{c}", name=f"W{c}")
            nc.sync.dma_start(out=wt, in_=ins["W"][:, c * WCH : (c + 1) * WCH])
            W_t.append(wt)

        T_sb = consts.tile([128, D * m], BF16, tag="T")  # T[e, k*m + i]

        # stats staging
        S_all = consts.tile([128, L], F32, tag="Sall")  # rows 0..m-1: var -> s
        S_bf = consts.tile([128, L], BF16, tag="Sbf")
        s_dram = dram.tile([m, L], BF16, tag="sdram")

        # ---- step 0: tmp[i,k,e] for all rows
        KB = max(1, 512 // m)  # k's per psum bank
        while D % KB:
            KB -= 1
        BANKS2 = 1024 // 512  # big tiles are 2 banks
        for kb2 in range(D // (KB * BANKS2)):
            ps0 = ps_big.tile([128, 1024], F32, tag="big")
            for half in range(BANKS2):
                kb = kb2 * BANKS2 + half
                for kk in range(KB):
                    k = kb * KB + kk
                    c, off = divmod(k * D, WCH)
                    nc.tensor.matmul(
                        ps0[:, 512 * half + kk * m : 512 * half + kk * m + m],
                        W_t[c][:, off : off + D],
                        XcT_sb[:, :m],
                        start=True,
                        stop=True,
                    )
            # copy both banks in one strided op per engine
            v = ps0.rearrange("p (h c) -> p h c", c=512)[:, :2, : KB * m]
            dstv = T_sb[:, kb2 * BANKS2 * KB * m : (kb2 + 1) * BANKS2 * KB * m]
            dstv = dstv.rearrange("p (h c) -> p h c", c=KB * m)
            if kb2 % 2 == 0:
                nc.scalar.activation(dstv, v, AF.Copy)
            else:
                nc.vector.tensor_copy(dstv, v)

        # pair_c row-pairs persist across the two passes (alloc after W release)
        wpool.release()
        prowp = es.enter_context(tc.tile_pool(name="prowp", bufs=1))
        prow2 = [
            prowp.tile([128, 2 * RS], BF16, tag=f"prow{i}", name=f"prow{i}")
            for i in range(npairs)
        ]

        T_k_i = T_sb.rearrange("p (k i) -> p k i", i=m)

        def split_copy2(dst2, src_ps2, bias=None):
            """Two-row PSUM->SBUF copy (+optional per-partition bias),
            front columns on ScalarE, back on DVE."""
            sv = _rows2(src_ps2, L, 512)
            dv = _rows2(dst2, L, RS)
            if bias is not None:
                nc.scalar.activation(
                    dv[:, :, :LH], sv[:, :, :LH], AF.Identity, bias=bias, scale=1.0
                )
                nc.vector.tensor_scalar(
                    dv[:, :, LH:L], sv[:, :, LH:L], bias, None, op0=ALU.add
                )
            else:
                nc.scalar.activation(dv[:, :, :LH], sv[:, :, :LH], AF.Copy)
                nc.vector.tensor_copy(dv[:, :, LH:L], sv[:, :, LH:L])

        # ---- pass A: pair rows + variance
        for g in range(ngroups):
            var_ps = ps_small.tile([128, 512], F32, tag="small")
            pair_ps2 = []
            for p in range(2):
                pp = ps_big.tile([128, 1024], F32, tag="big")
                for r in range(2):
                    i = GROUP * g + 2 * p + r
                    nc.tensor.matmul(
                        pp[:, 512 * r : 512 * r + L],
                        T_k_i[:, :, i],
                        xT_sb[:, :L],
                        start=True,
                        stop=True,
                    )
                pair_ps2.append(pp)
            pair22 = []
            for p in range(2):
                i2 = 2 * g + p
                split_copy2(prow2[i2], pair_ps2[p], bias=bsh_sb)
                p2 = sb.tile([128, 2 * RS], BF16, tag="pair2")
                if p == 0:
                    nc.gpsimd.tensor_mul(
                        _rows2(p2, L, RS), _rows2(prow2[i2], L, RS),
                        _rows2(prow2[i2], L, RS),
                    )
                else:
                    # (pair0 + bsh) * pair_biased == pair^2; reading PSUM for
                    # one operand avoids the DVE same-address port conflict
                    nc.vector.scalar_tensor_tensor(
                        _rows2(p2, L, RS), _rows2(pair_ps2[p], L, 512),
                        bsh_sb, _rows2(prow2[i2], L, RS),
                        op0=ALU.add, op1=ALU.mult,
                    )
                pair22.append(p2)
            for r in range(GROUP):
                nc.tensor.matmul(
                    var_ps[32 * r : 32 * (r + 1), :L],
                    ones_sb,
                    pair22[r // 2][:, RS * (r % 2) : RS * (r % 2) + L],
                    start=True,
                    stop=True,
                    tile_position=(0, 32 * r),
                )
            var_sb = sb.tile([128, L], F32, tag="var_sb")
            nc.scalar.activation(var_sb[:, :LH], var_ps[:, :LH], AF.Copy)
            nc.vector.tensor_copy(var_sb[:, LH:L], var_ps[:, LH:L])
            nc.gpsimd.dma_start(
                out=S_all[GROUP * g : GROUP * (g + 1), :L],
                in_=_p32(var_sb, GROUP, L),
            )

        # ---- stats (once): std = sqrt(var/128 + eps); s = 1/std; to DRAM
        nc.scalar.activation(
            S_all[:m, :L], S_all[:m, :L], AF.Sqrt, bias=eps_sb[:m], scale=1.0 / D
        )
        nc.vector.reciprocal_approx_fast(S_all[:m, :L], S_all[:m, :L])
        nc.vector.tensor_copy(S_bf[:m, :L], S_all[:m, :L])
        nc.sync.dma_start(out=s_dram[:, :], in_=S_bf[:m, :L])

        # ---- pass B: scale, MLP, contact rows
        for g in range(ngroups):
            out_ps = ps_small.tile([128, 512], F32, tag="small")
            srep4 = sbh.tile([128, GROUP * L], BF16, tag="srep")
            rows = s_dram[GROUP * g : GROUP * (g + 1), :]
            bcast = bass.AP(
                tensor=rows.tensor, offset=rows.offset, ap=[[0, 128], [1, GROUP * L]]
            )
            nc.sync.dma_start(out=srep4, in_=bcast)
            h_tiles = []
            for p in range(2):
                i2 = 2 * g + p
                h = sbh.tile([128, 2 * RS], BF16, tag="h")
                sview = srep4.rearrange("q (r c) -> q r c", c=L)[:, 2 * p : 2 * p + 2, :]
                nc.vector.tensor_mul(_rows2(h, L, RS), _rows2(prow2[i2], L, RS), sview)
                h_tiles.append(h)
            y_ps2 = []
            for p in range(2):
                yp = ps_big.tile([128, 1024], F32, tag="big")
                for r in range(2):
                    nc.tensor.matmul(
                        yp[:, 512 * r : 512 * r + L],
                        Wg_sb,
                        h_tiles[p][:, RS * r : RS * r + L],
                        start=True,
                        stop=True,
                    )
                y_ps2.append(yp)
            h2_tiles = []
            for p in range(2):
                h2 = sbh.tile([128, 2 * RS], BF16, tag="h2")
                nc.scalar.activation(
                    _rows2(h2, L, RS), _rows2(y_ps2[p], L, 512), AF.Gelu,
                    bias=cvec_sb, scale=1.0,
                )
                h2_tiles.append(h2)
            for r in range(GROUP):
                nc.tensor.matmul(
                    out_ps[32 * r : 32 * (r + 1), :L],
                    w2r_sb,
                    h2_tiles[r // 2][:, RS * (r % 2) : RS * (r % 2) + L],
                    start=True,
                    stop=True,
                    tile_position=(0, 32 * r),
                )
            orow = sb.tile([128, L], F32, tag="orow")
            nc.vector.tensor_copy(orow[:, :L], out_ps[:, :L])
            nc.gpsimd.dma_start(
                out=out_ap[GROUP * g : GROUP * (g + 1), :],
                in_=_p32(orow, GROUP, L),
            )


def build_nc(ngroups=NG_FULL):
    m = GROUP * ngroups
    nc = bacc.Bacc("TRN2", debug=False)
    ins = {
        "xT": nc.dram_tensor("xT", [D, L], BF16, kind="ExternalInput").ap(),
        "XcT": nc.dram_tensor("XcT", [D, m], BF16, kind="ExternalInput").ap(),
        "W": nc.dram_tensor("W", [D, D * D], BF16, kind="ExternalInput").ap(),
        "bsh": nc.dram_tensor("bsh", [D, 1], F32, kind="ExternalInput").ap(),
        "cvec": nc.dram_tensor("cvec", [D, 1], F32, kind="ExternalInput").ap(),
        "Wg": nc.dram_tensor("Wg", [D, D], BF16, kind="ExternalInput").ap(),
        "w2": nc.dram_tensor("w2", [D, 32], BF16, kind="ExternalInput").ap(),
    }
    out = nc.dram_tensor("out", [m, L], F32, kind="ExternalOutput").ap()
    with tile.TileContext(nc) as tc:
        kernel_body(tc, ins, out, ngroups)
    nc.compile()
    return nc


def host_prep(x, W_bil, b_bil, ln_g, ln_b, W1, b1, w2, b2):
    """Fold weights on host; build the 8 per-core input maps."""
    x = np.asarray(x, np.float32)
    W_bil = np.asarray(W_bil, np.float32)
    b_bil = np.asarray(b_bil, np.float32)
    ln_g = np.asarray(ln_g, np.float32)
    ln_b = np.asarray(ln_b, np.float32)
    W1 = np.asarray(W1, np.float32)
    b1 = np.asarray(b1, np.float32)
    w2 = np.asarray(w2, np.float32)

    Wc = W_bil - W_bil.mean(axis=0, keepdims=True)  # (k,d,e)
    W_host = np.ascontiguousarray(Wc.transpose(1, 0, 2).reshape(D, D * D)).astype(
        npbf16
    )
    bsh = (b_bil - b_bil.mean()).reshape(D, 1).astype(np.float32)
    Wg = np.ascontiguousarray((W1 * ln_g[None, :]).T).astype(npbf16)  # (k, e2)
    cvec = (W1 @ ln_b + b1).reshape(D, 1).astype(np.float32)
    w2c = np.ascontiguousarray(np.repeat(w2.reshape(D, 1), 32, axis=1)).astype(npbf16)

    xT = [np.ascontiguousarray(x[b].T).astype(npbf16) for b in range(B)]  # (D, L)

    in_maps = []
    for c in range(NCORES):
        b, s = c // 4, STARTS[c % 4]
        xc = np.zeros((M_FULL, D), np.float32)
        xc[:ROWS_VALID] = x[b, s : s + ROWS_VALID]
        in_maps.append(
            {
                "xT": xT[b],
                "XcT": np.ascontiguousarray(xc.T).astype(npbf16),
                "W": W_host,
                "bsh": bsh,
                "cvec": cvec,
                "Wg": Wg,
                "w2": w2c,
            }
        )
    return in_maps


def assemble(results, b2):
    """Gather per-core row blocks into the full symmetrized output."""
    contact = np.empty((B, L, L), np.float32)
    for c in range(NCORES):
        b, s = c // 4, STARTS[c % 4]
        contact[b, s : s + ROWS_VALID, :] = results[c]["out"][:ROWS_VALID]
    contact += np.float32(np.asarray(b2, np.float32).reshape(-1)[0])
    return (0.5 * (contact + contact.transpose(0, 2, 1))).astype(np.float32)


_NC_CACHE = {}


def _get_nc():
    if "nc" not in _NC_CACHE:
        _NC_CACHE["nc"] = build_nc(NG_FULL)
    return _NC_CACHE["nc"]


def run_on_device(in_maps, trace=False):
    from concourse.bass_utils import run_bass_kernel_spmd

    nc = _get_nc()
    return run_bass_kernel_spmd(
        nc, in_maps, core_ids=list(range(NCORES)), trace=trace
    )


def kernel(x, W_bil, b_bil, ln_g, ln_b, W1, b1, w2, b2):
    in_maps = host_prep(x, W_bil, b_bil, ln_g, ln_b, W1, b1, w2, b2)
    res = run_on_device(in_maps, trace=False)
    return assemble(res.results, b2)


# revision 21
# speedup vs baseline: 1.5466x; 1.1109x over previous
"""ContactMapHead Trainium2 kernel (8-way sharded, Bass/Tile).

Problem shapes (hardcoded): B=2, L=401, D=128.

Math (reference):
  pair[b,i,j,k] = x[b,i,:] @ W_bil[k] @ x[b,j,:] + b_bil[k]
  h  = LayerNorm_k(pair) * ln_g + ln_b
  y  = GELU(h @ W1.T + b1)
  contact[b,i,j] = y @ w2 + b2 ;  out = 0.5*(contact + contact^T)

Host folding:
  - W_bil centered over k  -> pair mean over k == 0, so LN = pair * rsqrt(mean(pair^2)+eps)
  - Wg[k,e] = W1[e,k]*ln_g[k],  cvec[e] = W1 @ ln_b + b1  (rides the GELU bias)
  - b2 and the symmetrization are applied on host (O(L^2), trivial)

Sharding: row-parallel. Core c (of 8) handles batch c//4, rows s..s+101
(s in {0,100,200,300}), padded to M=104 rows.

Device pipeline per core, (k x j) layout, bf16 matmuls, two passes so the
ScalarE activation table never thrashes between Sqrt and Gelu. Elementwise
ops batch TWO rows per instruction (sequencer issue overhead dominates at
one-row granularity). PSUM->SBUF copies are split ScalarE-front/DVE-back.
M=1 reduction matmuls run as M=32 col-strip tiles so PSUM partitions are
fully initialized for full-tile copies.
"""

import numpy as np
import ml_dtypes

import concourse.bass as bass
import concourse.tile as tile
from concourse import bacc, mybir

B, L, D = 2, 401, 128
NCORES = 8
GROUP = 4
NG_FULL = 26
M_FULL = GROUP * NG_FULL  # 104 padded rows per core
ROWS_VALID = 101
STARTS = (0, 100, 200, 300)

BF16 = mybir.dt.bfloat16
F32 = mybir.dt.float32
npbf16 = ml_dtypes.bfloat16
AF = mybir.ActivationFunctionType
ALU = mybir.AluOpType

RS = 416  # row stride (elements) inside two-row SBUF tiles (4B-aligned bf16)
LH = 256  # ScalarE/DVE split point for PSUM->SBUF copies


def _p32(ap_tile, nrows, ncols):
    """View of a [128, ...] tile exposing rows at partitions 0,32,64,96."""
    v = ap_tile.rearrange("(a b) f -> a b f", b=32)
    return v[:nrows, 0, :ncols]


def _rows2(t, cols, stride):
    """(128, 2, cols) view of a two-row tile with given row stride."""
    v = t.rearrange("p (r c) -> p r c", c=stride)
    return v[:, :2, :cols]


def kernel_body(tc, ins, out_ap, ngroups):
    nc = tc.nc
    m = GROUP * ngroups
    npairs = m // 2
    from contextlib import ExitStack

    with ExitStack() as es:
        consts = es.enter_context(tc.tile_pool(name="consts", bufs=1))
        sb = es.enter_context(tc.tile_pool(name="sb", bufs=4))
        sbh = es.enter_context(tc.tile_pool(name="sbh", bufs=4))
        ps_big = es.enter_context(tc.tile_pool(name="ps_big", bufs=3, space="PSUM"))
        ps_small = es.enter_context(tc.tile_pool(name="ps_small", bufs=2, space="PSUM"))
        dram = es.enter_context(tc.tile_pool(name="dram", bufs=1, space="DRAM"))

        # ---- constants / inputs to SBUF
        xT_sb = consts.tile([128, L], BF16, tag="xT")
        nc.sync.dma_start(out=xT_sb, in_=ins["xT"])
        XcT_sb = consts.tile([128, m], BF16, tag="XcT")
        nc.sync.dma_start(out=XcT_sb, in_=ins["XcT"])
        bsh_sb = consts.tile([128, 1], F32, tag="bsh")
        nc.sync.dma_start(out=bsh_sb, in_=ins["bsh"])
        cvec_sb = consts.tile([128, 1], F32, tag="cvec")
        nc.sync.dma_start(out=cvec_sb, in_=ins["cvec"])
        Wg_sb = consts.tile([128, D], BF16, tag="Wg")
        nc.sync.dma_start(out=Wg_sb, in_=ins["Wg"])
        ones_sb = consts.tile([128, 32], BF16, tag="ones")
        nc.vector.memset(ones_sb, 1.0)
        w2r_sb = consts.tile([128, 32], BF16, tag="w2r")
        nc.sync.dma_start(out=w2r_sb, in_=ins["w2"])
        eps_sb = consts.tile([128, 1], F32, tag="eps")
        nc.vector.memset(eps_sb, 1e-5)

        # W (d x k*e), 8 chunks so step0 can start before the full 4MB lands.
        # Own pool (created last = top of pool stack), released after step0
        # to give the SBUF back.
        wpool = tc.alloc_tile_pool(name="wpool", bufs=1)
        WCH = 2048
        W_t = []
        for c in range(D * D // WCH):
            wt = wpool.tile([128, WCH], BF16, tag=f"W{c}", name=f"W{c}")
            nc.sync.dma_start(out=wt, in_=ins["W"][:, c * WCH : (c + 1) * WCH])
            W_t.append(wt)

        T_sb = consts.tile([128, D * m], BF16, tag="T")  # T[e, k*m + i]

        # stats staging
        S_all = consts.tile([128, L], F32, tag="Sall")  # rows 0..m-1: var -> s
        S_bf = consts.tile([128, L], BF16, tag="Sbf")
        s_dram = dram.tile([m, L], BF16, tag="sdram")

        # ---- step 0: tmp[i,k,e] for all rows
        KB = max(1, 512 // m)  # k's per psum bank
        while D % KB:
            KB -= 1
        BANKS2 = 1024 // 512  # big tiles are 2 banks
        for kb2 in range(D // (KB * BANKS2)):
            ps0 = ps_big.tile([128, 1024], F32, tag="big")
            for half in range(BANKS2):
                kb = kb2 * BANKS2 + half
                for kk in range(KB):
                    k = kb * KB + kk
                    c, off = divmod(k * D, WCH)
                    nc.tensor.matmul(
                        ps0[:, 512 * half + kk * m : 512 * half + kk * m + m],
                        W_t[c][:, off : off + D],
                        XcT_sb[:, :m],
                        start=True,
                        stop=True,
                    )
            # copy both banks in one strided op per engine
            v = ps0.rearrange("p (h c) -> p h c", c=512)[:, :2, : KB * m]
            dstv = T_sb[:, kb2 * BANKS2 * KB * m : (kb2 + 1) * BANKS2 * KB * m]
            dstv = dstv.rearrange("p (h c) -> p h c", c=KB * m)
            if kb2 % 2 == 0:
                nc.scalar.activation(dstv, v, AF.Copy)
            else:
                nc.vector.tensor_copy(dstv, v)

        # pair_c row-pairs persist across the two passes (alloc after W release)
        wpool.release()
        prowp = es.enter_context(tc.tile_pool(name="prowp", bufs=1))
        prow2 = [
            prowp.tile([128, 2 * RS], BF16, tag=f"prow{i}", name=f"prow{i}")
            for i in range(npairs)
        ]

        T_k_i = T_sb.rearrange("p (k i) -> p k i", i=m)

        def split_copy2(dst2, src_ps2, bias=None):
            """Two-row PSUM->SBUF copy (+optional per-partition bias),
            front columns on ScalarE, back on DVE."""
            sv = _rows2(src_ps2, L, 512)
            dv = _rows2(dst2, L, RS)
            if bias is not None:
                nc.scalar.activation(
                    dv[:, :, :LH], sv[:, :, :LH], AF.Identity, bias=bias, scale=1.0
                )
                nc.vector.tensor_scalar(
                    dv[:, :, LH:L], sv[:, :, LH:L], bias, None, op0=ALU.add
                )
            else:
                nc.scalar.activation(dv[:, :, :LH], sv[:, :, :LH], AF.Copy)
                nc.vector.tensor_copy(dv[:, :, LH:L], sv[:, :, LH:L])

        # ---- pass A: pair rows + variance
        for g in range(ngroups):
            var_ps = ps_small.tile([128, 512], F32, tag="small")
            pair_ps2 = []
            for p in range(2):
                pp = ps_big.tile([128, 1024], F32, tag="big")
                for r in range(2):
                    i = GROUP * g + 2 * p + r
                    nc.tensor.matmul(
                        pp[:, 512 * r : 512 * r + L],
                        T_k_i[:, :, i],
                        xT_sb[:, :L],
                        start=True,
                        stop=True,
                    )
                pair_ps2.append(pp)
            pair22 = []
            for p in range(2):
                i2 = 2 * g + p
                split_copy2(prow2[i2], pair_ps2[p], bias=bsh_sb)
                p2 = sb.tile([128, 2 * RS], BF16, tag="pair2")
                if p == 0:
                    nc.gpsimd.tensor_mul(
                        _rows2(p2, L, RS), _rows2(prow2[i2], L, RS),
                        _rows2(prow2[i2], L, RS),
                    )
                else:
                    # (pair0 + bsh) * pair_biased == pair^2; reading PSUM for
                    # one operand avoids the DVE same-address port conflict
                    nc.vector.scalar_tensor_tensor(
                        _rows2(p2, L, RS), _rows2(pair_ps2[p], L, 512),
                        bsh_sb, _rows2(prow2[i2], L, RS),
                        op0=ALU.add, op1=ALU.mult,
                    )
                pair22.append(p2)
            for r in range(GROUP):
                nc.tensor.matmul(
                    var_ps[32 * r : 32 * (r + 1), :L],
                    ones_sb,
                    pair22[r // 2][:, RS * (r % 2) : RS * (r % 2) + L],
                    start=True,
                    stop=True,
                    tile_position=(0, 32 * r),
                )
            var_sb = sb.tile([128, L], F32, tag="var_sb")
            nc.scalar.activation(var_sb[:, :L], var_ps[:, :L], AF.Copy)
            nc.gpsimd.dma_start(
                out=S_all[GROUP * g : GROUP * (g + 1), :L],
                in_=_p32(var_sb, GROUP, L),
            )

        # ---- stats (once): std = sqrt(var/128 + eps); s = 1/std; to DRAM
        nc.scalar.activation(
            S_all[:m, :L], S_all[:m, :L], AF.Sqrt, bias=eps_sb[:m], scale=1.0 / D
        )
        nc.vector.reciprocal_approx_fast(S_all[:m, :L], S_all[:m, :L])
        nc.vector.tensor_copy(S_bf[:m, :L], S_all[:m, :L])
        nc.sync.dma_start(out=s_dram[:, :], in_=S_bf[:m, :L])

        # ---- pass B: scale, MLP, contact rows
        for g in range(ngroups):
            out_ps = ps_small.tile([128, 512], F32, tag="small")
            srep4 = sbh.tile([128, GROUP * L], BF16, tag="srep")
            rows = s_dram[GROUP * g : GROUP * (g + 1), :]
            bcast = bass.AP(
                tensor=rows.tensor, offset=rows.offset, ap=[[0, 128], [1, GROUP * L]]
            )
            nc.sync.dma_start(out=srep4, in_=bcast)
            h_tiles = []
            for p in range(2):
                i2 = 2 * g + p
                h = sbh.tile([128, 2 * RS], BF16, tag="h")
                sview = srep4.rearrange("q (r c) -> q r c", c=L)[:, 2 * p : 2 * p + 2, :]
                nc.vector.tensor_mul(_rows2(h, L, RS), _rows2(prow2[i2], L, RS), sview)
                h_tiles.append(h)
            y_ps2 = []
            for p in range(2):
                yp = ps_big.tile([128, 1024], F32, tag="big")
                for r in range(2):
                    nc.tensor.matmul(
                        yp[:, 512 * r : 512 * r + L],
                        Wg_sb,
                        h_tiles[p][:, RS * r : RS * r + L],
                        start=True,
                        stop=True,
                    )
                y_ps2.append(yp)
            h2_tiles = []
            for p in range(2):
                h2 = sbh.tile([128, 2 * RS], BF16, tag="h2")
                nc.scalar.activation(
                    _rows2(h2, L, RS), _rows2(y_ps2[p], L, 512), AF.Gelu,
                    bias=cvec_sb, scale=1.0,
                )
                h2_tiles.append(h2)
            for r in range(GROUP):
                nc.tensor.matmul(
                    out_ps[32 * r : 32 * (r + 1), :L],
                    w2r_sb,
                    h2_tiles[r // 2][:, RS * (r % 2) : RS * (r % 2) + L],
                    start=True,
                    stop=True,
                    tile_position=(0, 32 * r),
                )
            orow = sb.tile([128, L], F32, tag="orow")
            nc.vector.tensor_copy(orow[:, :L], out_ps[:, :L])
            nc.gpsimd.dma_start(
                out=out_ap[GROUP * g : GROUP * (g + 1), :],
                in_=_p32(orow, GROUP, L),
            )


def build_nc(ngroups=NG_FULL):
    m = GROUP * ngroups
    nc = bacc.Bacc("TRN2", debug=False)
    ins = {
        "xT": nc.dram_tensor("xT", [D, L], BF16, kind="ExternalInput").ap(),
        "XcT": nc.dram_tensor("XcT", [D, m], BF16, kind="ExternalInput").ap(),
        "W": nc.dram_tensor("W", [D, D * D], BF16, kind="ExternalInput").ap(),
        "bsh": nc.dram_tensor("bsh", [D, 1], F32, kind="ExternalInput").ap(),
        "cvec": nc.dram_tensor("cvec", [D, 1], F32, kind="ExternalInput").ap(),
        "Wg": nc.dram_tensor("Wg", [D, D], BF16, kind="ExternalInput").ap(),
        "w2": nc.dram_tensor("w2", [D, 32], BF16, kind="ExternalInput").ap(),
    }
    out = nc.dram_tensor("out", [m, L], F32, kind="ExternalOutput").ap()
    with tile.TileContext(nc) as tc:
        kernel_body(tc, ins, out, ngroups)
    nc.compile()
    return nc


def host_prep(x, W_bil, b_bil, ln_g, ln_b, W1, b1, w2, b2):
    """Fold weights on host; build the 8 per-core input maps."""
    x = np.asarray(x, np.float32)
    W_bil = np.asarray(W_bil, np.float32)
    b_bil = np.asarray(b_bil, np.float32)
    ln_g = np.asarray(ln_g, np.float32)
    ln_b = np.asarray(ln_b, np.float32)
    W1 = np.asarray(W1, np.float32)
    b1 = np.asarray(b1, np.float32)
    w2 = np.asarray(w2, np.float32)

    Wc = W_bil - W_bil.mean(axis=0, keepdims=True)  # (k,d,e)
    W_host = np.ascontiguousarray(Wc.transpose(1, 0, 2).reshape(D, D * D)).astype(
        npbf16
    )
    bsh = (b_bil - b_bil.mean()).reshape(D, 1).astype(np.float32)
    Wg = np.ascontiguousarray((W1 * ln_g[None, :]).T).astype(npbf16)  # (k, e2)
    cvec = (W1 @ ln_b + b1).reshape(D, 1).astype(np.float32)
    w2c = np.ascontiguousarray(np.repeat(w2.reshape(D, 1), 32, axis=1)).astype(npbf16)

    xT = [np.ascontiguousarray(x[b].T).astype(npbf16) for b in range(B)]  # (D, L)

    in_maps = []
    for c in range(NCORES):
        b, s = c // 4, STARTS[c % 4]
        xc = np.zeros((M_FULL, D), np.float32)
        xc[:ROWS_VALID] = x[b, s : s + ROWS_VALID]
        in_maps.append(
            {
                "xT": xT[b],
                "XcT": np.ascontiguousarray(xc.T).astype(npbf16),
                "W": W_host,
                "bsh": bsh,
                "cvec": cvec,
                "Wg": Wg,
                "w2": w2c,
            }
        )
    return in_maps


def assemble(results, b2):
    """Gather per-core row blocks into the full symmetrized output."""
    contact = np.empty((B, L, L), np.float32)
    for c in range(NCORES):
        b, s = c // 4, STARTS[c % 4]
        contact[b, s : s + ROWS_VALID, :] = results[c]["out"][:ROWS_VALID]
    contact += np.float32(np.asarray(b2, np.float32).reshape(-1)[0])
    return (0.5 * (contact + contact.transpose(0, 2, 1))).astype(np.float32)


_NC_CACHE = {}


def _get_nc():
    if "nc" not in _NC_CACHE:
        _NC_CACHE["nc"] = build_nc(NG_FULL)
    return _NC_CACHE["nc"]


def run_on_device(in_maps, trace=False):
    from concourse.bass_utils import run_bass_kernel_spmd

    nc = _get_nc()
    return run_bass_kernel_spmd(
        nc, in_maps, core_ids=list(range(NCORES)), trace=trace
    )


def kernel(x, W_bil, b_bil, ln_g, ln_b, W1, b1, w2, b2):
    in_maps = host_prep(x, W_bil, b_bil, ln_g, ln_b, W1, b1, w2, b2)
    res = run_on_device(in_maps, trace=False)
    return assemble(res.results, b2)
